# revision 100
# baseline (speedup 1.0000x reference)
"""Distributed Bass kernel for nn_Attention (B=8, S=1024, H=768, nh=12).

Sharding: data-parallel over batch — core b computes batch element b.
No collectives needed. Host side shards + layout-permutes inputs.

Key algebra (host precomputes, per batch element):
  eb[i,k,q] = exp(att_bias[q,k,i]) * (1 - mask[q,k])   (bf16, 0 at masked)
  mB[k,q]   = mask[q,k]                                 (bf16)
  Wq' = Wq * d^-0.5, bq' = bq * d^-0.5                  (scale folded)
  bo' = bv @ Wo + bo                                    (V-bias folded: attn rows sum to 1)

Per-core pipeline (all bf16 data, f32 psum):
  QT/KT[j, s] via matmul(lhsT=W[c,jtile], rhs=hT[c, schunk])  (contraction c)
  VZ natural [S, 12*(64+1)] with a ones column per head (softmax denom Z).
  ps[k, q]  = KT_head^T QT_head  (raw scores, 64-contraction)
  e = exp(ps)              (ACT, PSUM -> SBUF bf16)
  t = e * eb_tile          (DVE, bf16 2x)
  pts = t + mB             (DVE/GPSIMD split, bf16)
  po[0:64] = sum_k V*pts; po[64] = Z    (contraction k)
  OUTT[head rows, q] = po[0:64] * bcast(1/Z)
  res[s, j] = matmul(lhsT=OUTT[c, stile], rhs=Wo[c, jchunk]) + bo'
"""
import sys
import functools
import numpy as np

sys.path.insert(0, "/opt/trn_rl_repo")

NH, D, S, H, P = 12, 64, 1024, 768, 128
NT = H // P          # 6 chunks of the hidden dim
ST = S // P          # 8 tiles of the sequence dim
SCALE = D ** -0.5    # 0.125


def _body(nc, tc, tile, mybir, dr, out_dram):
    f32 = mybir.dt.float32
    bf16 = mybir.dt.bfloat16
    AF = mybir.ActivationFunctionType
    from concourse import bass
    PSUM = bass.MemorySpace.PSUM

    from contextlib import ExitStack

    with ExitStack() as ctx:
        pool = lambda *a, **k: ctx.enter_context(tc.tile_pool(*a, **k))
        qt_pool = pool(name="qt", bufs=1)
        kt_pool = pool(name="kt", bufs=1)
        vz_pool = pool(name="vz", bufs=1)
        pt_pool = pool(name="pt", bufs=2)
        mb_pool = pool(name="mb", bufs=1)
        ot_pool = pool(name="ot", bufs=1)
        cst_pool = pool(name="cst", bufs=1)
        hp_pool = pool(name="hp", bufs=1)
        wst_pool = pool(name="wst", bufs=1)
        wo_pool = pool(name="wo", bufs=1)
        eb_pool = pool(name="ebp", bufs=4)
        et_pool = pool(name="etp", bufs=4)
        tt_pool = pool(name="ttp", bufs=4)
        rz_pool = pool(name="rz", bufs=4)
        res_pool = pool(name="res", bufs=2)
        pss_pool = pool(name="pss", bufs=3, space=PSUM)
        po_pool = pool(name="po", bufs=2, space=PSUM)
        psm_pool = po_pool
        QT = [qt_pool.tile([P, S], bf16, name=f"QT{t}") for t in range(NT)]
        KT = [kt_pool.tile([P, S], bf16, name=f"KT{t}") for t in range(NT)]
        VZ = [vz_pool.tile([P, NH * (D + 1)], bf16, name=f"VZ{t}") for t in range(ST)]
        MB = [mb_pool.tile([P, S], bf16, name=f"MB{t}") for t in range(ST)]
        OUTT = [ot_pool.tile([P, S], bf16, name=f"OUTT{t}") for t in range(NT)]
        hT2 = [hp_pool.tile([P, 2 * S], bf16, name=f"hT{j}") for j in range(3)]
        hT = [hT2[c // 2][:, (c % 2) * S : (c % 2 + 1) * S] for c in range(NT)]
        ones_row = cst_pool.tile([1, P], f32, name="ones_row")
        bqk = cst_pool.tile([P, 2 * NT], f32, name="bqk")
        bqs = bqk[:, 0:NT]
        bks = bqk[:, NT : 2 * NT]
        bo_bc = cst_pool.tile([P, H], f32, name="bo_bc")
        borow = cst_pool.tile([1, H], f32, name="borow")

        warm = cst_pool.tile([1, 1], f32, name="warm")
        nc.vector.memset(ones_row[:], 1.0)
        # warm-up: force the ACT function-table load at t~0 (it otherwise
        # fires lazily, delaying the first projection evacuation by ~1.3us)
        nc.scalar.activation(warm[:], ones_row[0:1, 0:1], AF.Exp)
        # single packed bias load [128, 12]: cols 0-5 = bq' chunks, 6-11 = bk
        nc.sync.dma_start(bqk[:], dr["bqk"])
        # weights as big row-chunk tiles (fewer, larger DMAs); interleave
        # hT/WK/WQ per chunk so qk_chunk(0)'s accumulation can start early
        WQ2 = [wst_pool.tile([P, 2 * H], bf16, name=f"WQ{j}") for j in range(3)]
        WK2 = [wst_pool.tile([P, 2 * H], bf16, name=f"WK{j}") for j in range(3)]
        WV2 = [wst_pool.tile([P, 2 * H], bf16, name=f"WV{j}") for j in range(3)]
        WQ = [WQ2[c // 2][:, (c % 2) * H : (c % 2 + 1) * H] for c in range(NT)]
        WK = [WK2[c // 2][:, (c % 2) * H : (c % 2 + 1) * H] for c in range(NT)]
        WV = [WV2[c // 2][:, (c % 2) * H : (c % 2 + 1) * H] for c in range(NT)]

        def dpack(dram, j):
            # rows [256j, 256j+256) as [128, 2, width]: (p, r, q) <- row 256j+r*128+p
            return dram[2 * P * j : 2 * P * (j + 1), :].rearrange(
                "(r p) q -> p r q", r=2
            )

        def spack(t):
            return t[:].rearrange("p (r q) -> p r q", r=2)

        for j in range(3):
            nc.sync.dma_start(spack(hT2[j]), dpack(dr["hT"], j))
            nc.sync.dma_start(spack(WK2[j]), dpack(dr["Wk"], j))
        for j in range(3):
            nc.sync.dma_start(spack(WQ2[j]), dpack(dr["Wq"], j))
        for j in range(3):
            nc.sync.dma_start(spack(WV2[j]), dpack(dr["Wv"], j))
        nc.sync.dma_start(borow[:], dr["bo"][:])

        # ---------------- Q/K projection chunk 0 (unblocks heads 0-1) -----
        def qk_chunk(t):
            for wtiles, btile, dst in ((WK, bks, KT), (WQ, bqs, QT)):
                for sc in range(2):
                    ps = psm_pool.tile([P, 512], f32, name="po")
                    for c in range(NT):
                        nc.tensor.matmul(
                            ps[:],
                            wtiles[c][:, t * P : (t + 1) * P],
                            hT[c][:, sc * 512 : (sc + 1) * 512],
                            start=(c == 0),
                            stop=(c == NT - 1),
                        )
                    nc.scalar.activation(
                        dst[t][:, sc * 512 : (sc + 1) * 512],
                        ps[:],
                        AF.Identity,
                        bias=btile[:, t : t + 1],
                    )

        qk_chunk(0)

        # eb as double-k-tiles [128, 2*S] (half the DMA issues); prefetch
        # head 0 so attention starts early
        def eb_load(i, kt):
            ebt2 = eb_pool.tile([P, 2 * S], bf16, name="ebt")
            nc.sync.dma_start(
                ebt2[:].rearrange("p (r q) -> p r q", r=2),
                dr["ebias"][i, kt * P : (kt + 2) * P, :].rearrange(
                    "(r p) q -> p r q", r=2
                ),
            )
            return ebt2

        eb_prefetch = {}
        for kt in (0, 2, 4):
            eb_prefetch[(0, kt)] = eb_load(0, kt)

        for kt in range(ST):
            nc.sync.dma_start(MB[kt][:], dr["maskB"][kt * P : (kt + 1) * P, :])

        # ---------------- V projection -> VZ (ones col per head) ----------
        for st in range(ST):
            ones_cols = VZ[st][:].rearrange("p (h c) -> p h c", c=65)[:, :, 64:65]
            nc.vector.memset(ones_cols, 1.0)
        for jc in range(2):
            for st in range(ST):
                ps = psm_pool.tile([P, 512], f32, name="po")
                for c in range(NT):
                    nc.tensor.matmul(
                        ps[:, 0:384],
                        hT[c][:, st * P : (st + 1) * P],
                        WV[c][:, jc * 384 : (jc + 1) * 384],
                        start=(c == 0),
                        stop=(c == NT - 1),
                    )
                dst = VZ[st][:, jc * 390 : (jc + 1) * 390].rearrange(
                    "p (h c) -> p h c", c=65
                )[:, :, 0:64]
                src = ps[:, 0:384].rearrange("p (h c) -> p h c", c=64)
                nc.vector.tensor_copy(dst, src)


        # ---------------- Q/K projections, remaining chunks ----------------
        for t in range(1, NT):
            qk_chunk(t)

        # bo broadcast tile [128, 768] via ones-column matmul (needed only by
        # the output projection; built here during the projection lull)
        psb_a = psm_pool.tile([P, 512], f32, name="po")
        nc.tensor.matmul(psb_a[:], ones_row[:], borow[0:1, 0:512], start=True, stop=True)
        psb_b = psm_pool.tile([P, 512], f32, name="po")
        nc.tensor.matmul(psb_b[:, 0:256], ones_row[:], borow[0:1, 512:768],
                         start=True, stop=True)
        nc.scalar.activation(bo_bc[:, 0:512], psb_a[:], AF.Copy)
        nc.scalar.activation(bo_bc[:, 512:768], psb_b[:, 0:256], AF.Copy)

        # ---------------- attention per head ----------------
        wo = [wo_pool.tile([P, H], bf16, name=f"wo{c}") for c in range(NT)]
        for i in range(NH):
            if i == 2:
                # Wo loads, queued behind the early-head eb stream
                for c in range(NT):
                    nc.sync.dma_start(wo[c][:], dr["Wo"][c * P : (c + 1) * P, :])
            ch, off = i // 2, (i % 2) * D
            pts = [pt_pool.tile([P, S], bf16, name=f"pt{kt}") for kt in range(ST)]
            for kt in range(ST):
                if kt % 2 == 0:
                    ebt2 = eb_prefetch.pop((i, kt), None)
                    if ebt2 is None:
                        ebt2 = eb_load(i, kt)
                ebt = ebt2[:, (kt % 2) * S : (kt % 2 + 1) * S]
                ps = pss_pool.tile([P, S], f32, name="pss")
                for qc in range(2):
                    nc.tensor.matmul(
                        ps[:, qc * 512 : (qc + 1) * 512],
                        KT[ch][off : off + D, kt * P : (kt + 1) * P],
                        QT[ch][off : off + D, qc * 512 : (qc + 1) * 512],
                        start=True,
                        stop=True,
                    )
                et = et_pool.tile([P, S], bf16, name="et")
                t1 = tt_pool.tile([P, S], bf16, name="t1")
                nc.scalar.activation(et[:], ps[:], AF.Exp)
                nc.vector.tensor_mul(t1[:], et[:], ebt)
                if kt in (0, 2, 4, 6):
                    nc.gpsimd.tensor_add(pts[kt][:], t1[:], MB[kt][:])
                else:
                    nc.vector.tensor_add(pts[kt][:], t1[:], MB[kt][:])
            for qc in range(2):
                po = po_pool.tile([D + 1, 512], f32, name="po")
                for kt in range(ST):
                    nc.tensor.matmul(
                        po[:],
                        VZ[kt][:, i * 65 : (i + 1) * 65],
                        pts[kt][:, qc * 512 : (qc + 1) * 512],
                        start=(kt == 0),
                        stop=(kt == ST - 1),
                    )
                rz = rz_pool.tile([1, 512], f32, name="rz")
                nc.vector.reciprocal(rz[:], po[D : D + 1, :])
                ou = rz_pool.tile([D, 512], f32, name="ou")
                nc.scalar.activation(ou[:], po[0:D, :], AF.Copy)
                pb = po_pool.tile([D, 512], f32, name="po")
                nc.tensor.matmul(pb[:], ones_row[0:1, 0:D], rz[:],
                                 start=True, stop=True)
                nc.vector.tensor_mul(
                    OUTT[ch][off : off + D, qc * 512 : (qc + 1) * 512],
                    pb[:],
                    ou[:],
                )

        # ---------------- output projection ----------------
        for st in range(ST):
            res = res_pool.tile([P, H], f32, name="res")
            for jc in range(2):
                ps = psm_pool.tile([P, 512], f32, name="po")
                for ch in range(NT):
                    nc.tensor.matmul(
                        ps[:, 0:384],
                        OUTT[ch][:, st * P : (st + 1) * P],
                        wo[ch][:, jc * 384 : (jc + 1) * 384],
                        start=(ch == 0),
                        stop=(ch == NT - 1),
                    )
                nc.vector.tensor_add(
                    res[:, jc * 384 : (jc + 1) * 384],
                    ps[:, 0:384],
                    bo_bc[:, jc * 384 : (jc + 1) * 384],
                )
            nc.sync.dma_start(out_dram[st * P : (st + 1) * P, :], res[:])


@functools.lru_cache(maxsize=1)
def _build():
    from concourse import bacc, tile, mybir

    nc = bacc.Bacc("TRN2", target_bir_lowering=False, debug=False, num_devices=8)
    f32 = mybir.dt.float32
    bf16 = mybir.dt.bfloat16
    dr = {
        "hT": nc.dram_tensor("hT", [H, S], bf16, kind="ExternalInput").ap(),
        "ebias": nc.dram_tensor("ebias", [NH, S, S], bf16, kind="ExternalInput").ap(),
        "maskB": nc.dram_tensor("maskB", [S, S], bf16, kind="ExternalInput").ap(),
    }
    for w in ("Wq", "Wk", "Wv", "Wo"):
        dr[w] = nc.dram_tensor(w, [H, H], bf16, kind="ExternalInput").ap()
    dr["bqk"] = nc.dram_tensor("bqk", [P, 2 * NT], f32, kind="ExternalInput").ap()
    dr["bo"] = nc.dram_tensor("bo", [H], f32, kind="ExternalInput").ap()
    out = nc.dram_tensor("out", [S, H], f32, kind="ExternalOutput").ap()

    with tile.TileContext(nc) as tc:
        _body(nc, tc, tile, mybir, dr, out)
    nc.compile()
    return nc


def make_in_maps(**inputs):
    import ml_dtypes
    bf = ml_dtypes.bfloat16
    h = np.asarray(inputs["h"], np.float32)
    ab = np.asarray(inputs["att_bias"], np.float32)
    mk = np.asarray(inputs["mask"], np.int32)
    Wq = np.asarray(inputs["Wq"], np.float32)
    Wk = np.asarray(inputs["Wk"], np.float32)
    Wv = np.asarray(inputs["Wv"], np.float32)
    Wo = np.asarray(inputs["Wo"], np.float32)
    bq = np.asarray(inputs["bq"], np.float32)
    bk = np.asarray(inputs["bk"], np.float32)
    bv = np.asarray(inputs["bv"], np.float32)
    bo = np.asarray(inputs["bo"], np.float32)

    shared = {
        "Wq": (Wq * SCALE).astype(bf),
        "Wk": Wk.astype(bf),
        "Wv": Wv.astype(bf),
        "Wo": Wo.astype(bf),
        "bqk": np.ascontiguousarray(np.concatenate(
            [(bq * SCALE).reshape(NT, P).T, bk.reshape(NT, P).T], axis=1
        )).astype(np.float32),
        "bo": (bv @ Wo + bo).astype(np.float32),
    }
    in_maps = []
    for b in range(8):
        m = dict(shared)
        m["hT"] = np.ascontiguousarray(h[b].T).astype(bf)
        mT = mk[b].T.astype(np.float32)          # [k, q]
        ebT = np.exp(ab[b].transpose(2, 1, 0))   # [nh, k, q]
        ebT *= (1.0 - mT)[None]
        m["ebias"] = ebT.astype(bf)
        m["maskB"] = mT.astype(bf)
        in_maps.append(m)
    return in_maps


def kernel(**inputs):
    nc = _build()
    from concourse import bass_utils

    in_maps = make_in_maps(**inputs)
    res = bass_utils.run_bass_kernel_spmd(nc, in_maps, core_ids=list(range(8)))
    return np.stack([r["out"] for r in res.results], axis=0)

